# revision 34
# baseline (speedup 1.0000x reference)
"""Trainium2 kernel for nn_AP (temporal-action-detection average precision).

Reference computation:
  - B=256 videos, N=4000 proposals, G=50 ground-truths, IoU thresholds (0.5, 0.75).
  - Per (video, thr): pot[n,g] = IoU(seg_n, gt_g) > thr; greedy matching over
    GT columns claims the first (lowest-index) unused candidate -> is_TP[B,N].
  - Global: sort all B*N scores desc, cumsum TP, AP = sum |dx| * cummax(y).

Device part (8 NeuronCores, data-parallel over B; 32 videos/core):
  Uses the identity  IoU > tau  <=>  la + lb - kinv*(|as-bs|+|ae-be|) > 0  with
  kinv = (1+tau)/(1-tau) in {3, 7}.  All value-carrying matmuls use exact
  two-term bf16 splits (x = bf16(x) + bf16(x - bf16(x)); bf16 products are
  exact in fp32 PSUM), so TensorE runs at bf16 rate with ~fp32 accuracy
  (margin error ~2^-18, far below the AP tolerance):
    - TensorE broadcasts per-video as/ae rows across the 100 (video-in-pair,
      GT) partitions and computes margin[(thr,g); n] = la + lb - kinv*u.
    - ScalarE computes t1=|as-bs|, t2=|ae-be| via Abs activation w/ per-GT
      bias, and the bf16 head of u.
    - GpSimd computes u = t1+t2 and the bf16 tail of u.
    - VectorE: enc = (margin > 0) * idxval  where idxval encodes the proposal
      index as 1-(j+1)*2^-19 (fp32-exact); max8(enc) returns the first 8
      candidate indices per (GT, thr) column (values decode to indices).
  Host part: tiny greedy over the first-8 lists (measured max conflict depth
  7), plus the global ranking of TP confidences (one sort) and the AP sum.
"""

import os
import numpy as np
import ml_dtypes

import concourse.bass as bass
import concourse.tile as tile
from concourse import bacc, mybir
from concourse.bass_utils import run_bass_kernel_spmd

# problem constants (hardcoded per spec nn_AP_19258633355825)
B, N, G = 256, 4000, 50
NCORES = 8
NV = B // NCORES          # videos per core
NP2 = NV // 2             # video pairs per core
NPAD = 4096               # padded proposal dim
HALF = 2048               # margin/enc processed in halves (PSUM capacity)
KINV = (3.0, 7.0)         # (1+tau)/(1-tau) for tau in (0.5, 0.75)
F32 = mybir.dt.float32
BF16 = mybir.dt.bfloat16
NPBF = ml_dtypes.bfloat16
PAD_VAL = 1.0e6           # sentinel start/end for padded proposals


def _split2(x):
    """Exact-ish 2-term bf16 split: x ~ h1 + h2 with |err| <= 2^-18 |x|."""
    h1 = x.astype(NPBF)
    h2 = (x - h1.astype(np.float32)).astype(NPBF)
    return h1, h2


# ----------------------------------------------------------------- constants
def _consts():
    # o8 [8, 200]: broadcast lhsT. stg rows per pair: (video r, kind k) with
    # kinds [as_h1, as_h2, ae_h1, ae_h2]. Columns: [0:100] -> as_rep,
    # [100:200] -> ae_rep; within, m = r*50 + g.
    o8 = np.zeros((8, 200), np.float32)
    for r in range(2):
        o8[4 * r + 0, 0 + r * 50:0 + r * 50 + 50] = 1.0    # as_h1
        o8[4 * r + 1, 0 + r * 50:0 + r * 50 + 50] = 1.0    # as_h2
        o8[4 * r + 2, 100 + r * 50:100 + r * 50 + 50] = 1.0  # ae_h1
        o8[4 * r + 3, 100 + r * 50:100 + r * 50 + 50] = 1.0  # ae_h2
    j = np.arange(HALF, dtype=np.float64)
    iv = (1.0 - (j + 1) * 2.0**-19).astype(np.float32)
    idxval = np.ascontiguousarray(np.broadcast_to(iv, (128, HALF)))
    return o8.astype(NPBF), idxval


def _core_inputs(seg, lab):
    """Host-side preprocessing for one core's shard (seg [NV,N,2], lab [NV,G,2])."""
    as_ = np.full((NV, NPAD), PAD_VAL, np.float32)
    ae = np.full((NV, NPAD), PAD_VAL, np.float32)
    as_[:, :N] = seg[:, :, 0]
    ae[:, :N] = seg[:, :, 1]
    la = np.zeros((NV, NPAD), np.float32)
    la[:, :N] = ae[:, :N] - as_[:, :N]

    sg4 = np.empty((NV, 4, NPAD), NPBF)
    sg4[:, 0], sg4[:, 1] = _split2(as_)
    sg4[:, 2], sg4[:, 3] = _split2(ae)
    laon = np.empty((NV, 4, NPAD), NPBF)
    laon[:, 0], laon[:, 1] = _split2(la)
    laon[:, 2] = NPBF(1.0)
    laon[:, 3] = NPBF(1.0)

    lb = (lab[:, :, 1] - lab[:, :, 0]).astype(np.float32)   # [NV, G]
    lbh1, lbh2 = _split2(lb)
    lh = np.zeros((NV, 106, 100), np.float32)
    for t, kinv in enumerate(KINV):
        for g in range(G):
            lh[0::2, g, t * 50 + g] = -kinv
            lh[1::2, 50 + g, t * 50 + g] = -kinv
    lh[0::2, 100, :] = 1.0
    lh[0::2, 101, :] = 1.0
    lh[1::2, 102, :] = 1.0
    lh[1::2, 103, :] = 1.0
    lh[:, 104, :] = np.tile(lbh1.astype(np.float32), (1, 2))
    lh[:, 105, :] = np.tile(lbh2.astype(np.float32), (1, 2))

    labc = np.empty((100, 2 * NP2), np.float32)
    for p in range(NP2):
        for r in range(2):
            labc[r * 50:r * 50 + 50, 2 * p] = lab[2 * p + r, :, 0]
            labc[r * 50:r * 50 + 50, 2 * p + 1] = lab[2 * p + r, :, 1]
    return {"sg4": sg4, "laon": laon, "lh": lh.astype(NPBF), "labc": labc}


# ----------------------------------------------------------------- device IR
def build_nc():
    nc = bacc.Bacc("TRN2", target_bir_lowering=False, debug=False,
                   num_devices=NCORES)

    sg4_d = nc.dram_tensor("sg4", [NV, 4, NPAD], BF16, kind="ExternalInput")
    laon_d = nc.dram_tensor("laon", [NV, 4, NPAD], BF16, kind="ExternalInput")
    lh_d = nc.dram_tensor("lh", [NV, 106, 100], BF16, kind="ExternalInput")
    labc_d = nc.dram_tensor("labc", [100, 2 * NP2], F32, kind="ExternalInput")
    o8_d = nc.dram_tensor("o8", [8, 200], BF16, kind="ExternalInput")
    idxv_d = nc.dram_tensor("idxv", [128, HALF], F32, kind="ExternalInput")
    out = nc.dram_tensor("out", [100, NV * 16], F32, kind="ExternalOutput")

    with tile.TileContext(nc) as tc:
        with (
            tc.tile_pool(name="const", bufs=1) as cpool,
            tc.tile_pool(name="stg", bufs=3) as stgp,
            tc.tile_pool(name="lht", bufs=4) as lhp,
            tc.tile_pool(name="t12", bufs=3) as t12p,
            tc.tile_pool(name="th", bufs=3) as thp,
            tc.tile_pool(name="enc", bufs=3) as encp,
            tc.tile_pool(name="ps_a", bufs=2, space="PSUM") as ps_a,
            tc.tile_pool(name="ps_e", bufs=2, space="PSUM") as ps_e,
            tc.tile_pool(name="ps_m", bufs=1, space="PSUM") as ps_m,
        ):
            # --- constants
            o8 = cpool.tile([8, 200], BF16)
            nc.sync.dma_start(o8[:], o8_d[:])
            idxv = cpool.tile([128, HALF], F32)
            nc.sync.dma_start(idxv[:], idxv_d[:])
            labc = cpool.tile([100, 2 * NP2], F32)
            nc.sync.dma_start(labc[:], labc_d[:])
            m8_all = cpool.tile([100, NV * 16], F32)

            def front(p):
                vA = 2 * p
                # staging: [8, NPAD] = both videos' 4 split rows
                stg = stgp.tile([8, NPAD], BF16)
                nc.sync.dma_start(
                    stg[:], sg4_d[vA:vA + 2].rearrange("v k n -> (v k) n"))

                # th1: rows 0..99 = bf16 head of u; 100..103 = la splits
                # (A_h1, A_h2, B_h1, B_h2); 104..105 = ones
                th1 = thp.tile([106, NPAD], BF16, tag="th1")
                nc.sync.dma_start(th1[100:102, :], laon_d[vA, 0:2])
                nc.sync.dma_start(th1[102:104, :], laon_d[vA + 1, 0:2])
                nc.sync.dma_start(th1[104:106, :], laon_d[vA, 2:4])

                # per-video margin lhsT
                lhA = lhp.tile([106, 100], BF16, tag="lhA")
                lhB = lhp.tile([106, 100], BF16, tag="lhB")
                nc.sync.dma_start(lhA[:], lh_d[vA])
                nc.sync.dma_start(lhB[:], lh_d[vA + 1])

                t1 = t12p.tile([100, NPAD], F32, tag="t1")
                t2 = t12p.tile([100, NPAD], F32, tag="t2")
                for c in range(NPAD // 512):
                    w = 512 if c < 7 else N - 7 * 512
                    cs = slice(c * 512, c * 512 + w)
                    a_ps = ps_a.tile([100, 512], F32)
                    nc.tensor.matmul(a_ps[:, 0:w], o8[:, 0:100], stg[:, cs],
                                     start=True, stop=True)
                    e_ps = ps_e.tile([100, 512], F32)
                    nc.tensor.matmul(e_ps[:, 0:w], o8[:, 100:200], stg[:, cs],
                                     start=True, stop=True)
                    nc.scalar.activation(t1[:, cs], a_ps[:, 0:w],
                                         mybir.ActivationFunctionType.Abs,
                                         bias=labc[:, 2 * p:2 * p + 1],
                                         scale=-1.0)
                    nc.scalar.activation(t2[:, cs], e_ps[:, 0:w],
                                         mybir.ActivationFunctionType.Abs,
                                         bias=labc[:, 2 * p + 1:2 * p + 2],
                                         scale=-1.0)
                # th1 head = bf16(t1 + t2) straight from gpsimd, per half
                for hh in range(2):
                    hs = slice(hh * HALF, HALF + hh * (N - HALF))
                    nc.gpsimd.tensor_tensor(th1[0:100, hs], t1[:, hs],
                                            t2[:, hs], mybir.AluOpType.add)

                return th1, lhA, lhB

            def back(p, th1, lhA, lhB):
                for v in range(2):
                    lht = lhA if v == 0 else lhB
                    for h in range(2):
                        hw = HALF if h == 0 else N - HALF
                        mps = ps_m.tile([100, HALF], F32)
                        for cc in range((hw + 511) // 512):
                            w = min(512, hw - cc * 512)
                            ns = slice(h * HALF + cc * 512,
                                       h * HALF + cc * 512 + w)
                            ms = slice(cc * 512, cc * 512 + w)
                            nc.tensor.matmul(mps[:, ms], lht[:],
                                             th1[:, ns],
                                             start=True, stop=True)
                        enc = encp.tile([100, HALF], F32, tag="enc")
                        nc.vector.scalar_tensor_tensor(
                            enc[:, 0:hw], mps[:, 0:hw], 0.0,
                            idxv[0:100, 0:hw],
                            op0=mybir.AluOpType.is_gt,
                            op1=mybir.AluOpType.mult)
                        col = ((p * 2 + v) * 2 + h) * 8
                        nc.vector.max(m8_all[:, col:col + 8], enc[:, 0:hw])

            state = front(0)
            for p in range(1, NP2):
                nxt = front(p)
                back(p - 1, *state)
                state = nxt
            back(NP2 - 1, *state)
            nc.sync.dma_start(out[:], m8_all[:])
    nc.compile()
    return nc


_NC_CACHE = None


def _get_nc():
    global _NC_CACHE
    if _NC_CACHE is None:
        _NC_CACHE = build_nc()
    return _NC_CACHE


# ------------------------------------------------------------------ host post
NQ = 2
QW = NPAD // NQ


def _decode_first8(dev_out):
    """dev_out: list of [100, NV*32] fp32 per core
    -> cand [B, 2(thr), G, 32] int64, fullq [B, 2, G, NQ]."""
    d = np.stack(dev_out)                                # [C, 100, NV*32]
    d = d.reshape(NCORES, 2, G, NV, NQ, 8)               # [C, t, g, vid, q, s]
    d = np.transpose(d, (0, 3, 1, 2, 4, 5))              # [C, vid, t, g, q, s]
    v = d.reshape(B, 2, G, NQ, 8)
    valid = v > 0.5
    j = np.rint((1.0 - v.astype(np.float64)) * 2.0**19).astype(np.int64) - 1
    off = (np.arange(NQ) * QW)[None, None, None, :, None]
    n = np.where(valid, j + off, -1)
    cand = n.reshape(B, 2, G, NQ * 8)
    cand = np.where((cand >= 0) & (cand < N), cand, -1)
    fullq = valid[:, :, :, :, 7]                         # [b, t, g, q]
    return cand, fullq


def _greedy(cand, fullq, segments, labels):
    """cand [B,2,G,NQ*8] first candidate idxs (-1 none) per (video,thr,GT).
    Returns is_tp [2, B, N] bool. Exact host fallback for >8-deep conflicts."""
    V = B * 2
    c = cand.reshape(V, G, NQ * 8)
    f8 = fullq.reshape(V, G, NQ)
    used = np.zeros((V, N), bool)
    rows = np.arange(V)
    for g in range(G):
        lst = c[:, g, :]                                 # [V, NQ*8]
        valid = lst >= 0
        taken = np.take_along_axis(used, np.clip(lst, 0, None), axis=1)
        free = valid & ~taken
        # fallback: quarter q fully-listed(8) with all its entries
        # used/invalid while deeper candidates may exist there.
        blocked = False
        for q in range(NQ):
            blocked = blocked | (f8[:, g, q]
                                 & ~free[:, 8 * q:8 * q + 8].any(1))
        fb = np.nonzero(blocked)[0]
        for i in fb:
            b, t = i // 2, i % 2
            kinv = np.float32(KINV[t])
            s = segments[b]
            uu = (np.abs(s[:, 0] - labels[b, g, 0])
                  + np.abs(s[:, 1] - labels[b, g, 1])).astype(np.float32)
            marg = ((s[:, 1] - s[:, 0]) +
                    (labels[b, g, 1] - labels[b, g, 0]) - kinv * uu)
            cnz = np.nonzero(marg.astype(np.float32) > 0)[0]
            fr = cnz[~used[i, cnz]]
            if len(fr):
                used[i, fr[0]] = True
            free[i] = False
        has = free.any(1)
        j = np.argmax(free, axis=1)
        sel = rows[has]
        used[sel, lst[sel, j[has]]] = True
    return used.reshape(B, 2, N).transpose(1, 0, 2)


def _ap_from_tp(is_tp, scores):
    """is_tp [2, B, N] bool, scores [B, N] -> AP [2] float32 (exact ranking)."""
    conf = scores.reshape(-1)
    M = conf.size
    bits = conf.view(np.uint32).astype(np.int64)
    key = (bits << 20) + (2**20 - 1 - np.arange(M, dtype=np.int64))
    skey = np.sort(key)
    out = np.empty(2, np.float32)
    for t in range(2):
        tp_idx = np.nonzero(is_tp[t].reshape(-1))[0]
        k = key[tp_idx]
        # rank (1-based) in descending order = #{keys > k} + 1
        r = np.sort(M - np.searchsorted(skey, k, side="left"))
        kk = np.arange(1, len(r) + 1, dtype=np.float64)
        prec = (kk / r).astype(np.float32)
        sufmax = np.maximum.accumulate(prec[::-1])[::-1]
        out[t] = np.float32(sufmax.astype(np.float64).sum() / (B * G))
    return out


def _enable_profiling():
    """Dev-only: register the NTFF profiling hook (missing antenv shim) and
    keep artifacts local. Returns extra kwargs for run_bass_kernel_spmd."""
    import sys
    import types
    import tempfile

    if "antenv.axon_hooks" not in sys.modules:
        mod = types.ModuleType("antenv.axon_hooks")
        _h = [None]
        mod.set_axon_ntff_profile_hook = lambda h: _h.__setitem__(0, h)
        mod.get_axon_ntff_profile_hook = lambda: _h[0]
        sys.modules["antenv.axon_hooks"] = mod
        from trn_agent_boot.trn_boot import _ntff_profile_via_ctypes
        mod.set_axon_ntff_profile_hook(
            _ntff_profile_via_ctypes("/opt/axon/libaxon_pjrt.so"))
    import concourse.bass_utils as bu
    bu.upload_artifacts = lambda tmpdir: tmpdir
    tdir = os.environ.get("ATH_TRACE_DIR") or tempfile.mkdtemp(
        prefix="ap_trace_")
    print("trace dir:", tdir)
    return {"tmpdir": tdir}


# ------------------------------------------------------------------- kernel
def kernel(scores, segments, labels):
    scores = np.ascontiguousarray(scores, np.float32)
    segments = np.ascontiguousarray(segments, np.float32)
    labels = np.ascontiguousarray(labels, np.float32)

    o8, idxval = _consts()
    in_maps = []
    for i in range(NCORES):
        sl = slice(i * NV, (i + 1) * NV)
        m = _core_inputs(segments[sl], labels[sl])
        m.update({"o8": o8, "idxv": idxval})
        in_maps.append(m)
    nc = _get_nc()
    trace = bool(int(os.environ.get("ATH_PROFILE", "0")))
    kw = {}
    if trace:
        try:
            kw = _enable_profiling()
        except Exception as e:           # profiling is best-effort
            print("profiling unavailable:", e)
            trace = False
    res = run_bass_kernel_spmd(nc, in_maps, core_ids=list(range(NCORES)),
                               trace=trace, **kw)
    if trace and res.exec_time_ns is not None:
        print(f"HW exec time: {res.exec_time_ns} ns")
    dev = [res.results[i]["out"] for i in range(NCORES)]

    cand, full8 = _decode_first8(dev)
    is_tp = _greedy(cand, full8, segments, labels)
    return _ap_from_tp(is_tp, scores)


# revision 35
# speedup vs baseline: 1.2247x; 1.2247x over previous
"""Trainium2 kernel for nn_AP (temporal-action-detection average precision).

Reference computation:
  - B=256 videos, N=4000 proposals, G=50 ground-truths, IoU thresholds (0.5, 0.75).
  - Per (video, thr): pot[n,g] = IoU(seg_n, gt_g) > thr; greedy matching over
    GT columns claims the first (lowest-index) unused candidate -> is_TP[B,N].
  - Global: sort all B*N scores desc, cumsum TP, AP = sum |dx| * cummax(y).

Device part (8 NeuronCores, data-parallel over B; 32 videos/core):
  Uses the identity  IoU > tau  <=>  la + lb - kinv*(|as-bs|+|ae-be|) > 0  with
  kinv = (1+tau)/(1-tau) in {3, 7}.  All value-carrying matmuls use exact
  two-term bf16 splits (x = bf16(x) + bf16(x - bf16(x)); bf16 products are
  exact in fp32 PSUM), so TensorE runs at bf16 rate with ~fp32 accuracy
  (margin error ~2^-18, far below the AP tolerance):
    - TensorE broadcasts per-video as/ae rows across the 100 (video-in-pair,
      GT) partitions and computes margin[(thr,g); n] = la + lb - kinv*u.
    - ScalarE computes t1=|as-bs|, t2=|ae-be| via Abs activation w/ per-GT
      bias, and the bf16 head of u.
    - GpSimd computes u = t1+t2 and the bf16 tail of u.
    - VectorE: enc = (margin > 0) * idxval  where idxval encodes the proposal
      index as 1-(j+1)*2^-19 (fp32-exact); max8(enc) returns the first 8
      candidate indices per (GT, thr) column (values decode to indices).
  Host part: tiny greedy over the first-8 lists (measured max conflict depth
  7), plus the global ranking of TP confidences (one sort) and the AP sum.
"""

import os
import numpy as np
import ml_dtypes

import concourse.bass as bass
import concourse.tile as tile
from concourse import bacc, mybir
from concourse.bass_utils import run_bass_kernel_spmd

# problem constants (hardcoded per spec nn_AP_19258633355825)
B, N, G = 256, 4000, 50
NCORES = 8
NV = B // NCORES          # videos per core
NP2 = NV // 2             # video pairs per core
NPAD = 4096               # padded proposal dim
HALF = 2048               # margin/enc processed in halves (PSUM capacity)
KINV = (3.0, 7.0)         # (1+tau)/(1-tau) for tau in (0.5, 0.75)
F32 = mybir.dt.float32
BF16 = mybir.dt.bfloat16
NPBF = ml_dtypes.bfloat16
PAD_VAL = 1.0e6           # sentinel start/end for padded proposals


def _split2(x):
    """Exact-ish 2-term bf16 split: x ~ h1 + h2 with |err| <= 2^-18 |x|."""
    h1 = x.astype(NPBF)
    h2 = (x - h1.astype(np.float32)).astype(NPBF)
    return h1, h2


# ----------------------------------------------------------------- constants
def _consts():
    # o8 [8, 200]: broadcast lhsT. stg rows per pair: (video r, kind k) with
    # kinds [as_h1, as_h2, ae_h1, ae_h2]. Columns: [0:100] -> as_rep,
    # [100:200] -> ae_rep; within, m = r*50 + g.
    o8 = np.zeros((8, 200), np.float32)
    for r in range(2):
        o8[4 * r + 0, 0 + r * 50:0 + r * 50 + 50] = 1.0    # as_h1
        o8[4 * r + 1, 0 + r * 50:0 + r * 50 + 50] = 1.0    # as_h2
        o8[4 * r + 2, 100 + r * 50:100 + r * 50 + 50] = 1.0  # ae_h1
        o8[4 * r + 3, 100 + r * 50:100 + r * 50 + 50] = 1.0  # ae_h2
    j = np.arange(HALF, dtype=np.float64)
    iv = (1.0 - (j + 1) * 2.0**-19).astype(np.float32)
    idxval = np.ascontiguousarray(np.broadcast_to(iv, (128, HALF)))
    return o8.astype(NPBF), idxval


def _core_inputs(seg, lab):
    """Host-side preprocessing for one core's shard (seg [NV,N,2], lab [NV,G,2])."""
    as_ = np.full((NV, NPAD), PAD_VAL, np.float32)
    ae = np.full((NV, NPAD), PAD_VAL, np.float32)
    as_[:, :N] = seg[:, :, 0]
    ae[:, :N] = seg[:, :, 1]
    la = np.zeros((NV, NPAD), np.float32)
    la[:, :N] = ae[:, :N] - as_[:, :N]

    sg4 = np.empty((NV, 4, NPAD), NPBF)
    sg4[:, 0], sg4[:, 1] = _split2(as_)
    sg4[:, 2], sg4[:, 3] = _split2(ae)
    laon = np.empty((NV, 4, NPAD), NPBF)
    laon[:, 0], laon[:, 1] = _split2(la)
    laon[:, 2] = NPBF(1.0)
    laon[:, 3] = NPBF(1.0)

    lb = (lab[:, :, 1] - lab[:, :, 0]).astype(np.float32)   # [NV, G]
    lbh1, lbh2 = _split2(lb)
    lh = np.zeros((NV, 106, 100), np.float32)
    for t, kinv in enumerate(KINV):
        for g in range(G):
            lh[0::2, g, t * 50 + g] = -kinv
            lh[1::2, 50 + g, t * 50 + g] = -kinv
    lh[0::2, 100, :] = 1.0
    lh[0::2, 101, :] = 1.0
    lh[1::2, 102, :] = 1.0
    lh[1::2, 103, :] = 1.0
    lh[:, 104, :] = np.tile(lbh1.astype(np.float32), (1, 2))
    lh[:, 105, :] = np.tile(lbh2.astype(np.float32), (1, 2))

    labc = np.empty((100, 2 * NP2), np.float32)
    for p in range(NP2):
        for r in range(2):
            labc[r * 50:r * 50 + 50, 2 * p] = lab[2 * p + r, :, 0]
            labc[r * 50:r * 50 + 50, 2 * p + 1] = lab[2 * p + r, :, 1]
    return {"sg4": sg4, "laon": laon, "lh": lh.astype(NPBF), "labc": labc}


# ----------------------------------------------------------------- device IR
def build_nc():
    nc = bacc.Bacc("TRN2", target_bir_lowering=False, debug=False,
                   num_devices=NCORES)

    sg4_d = nc.dram_tensor("sg4", [NV, 4, NPAD], BF16, kind="ExternalInput")
    laon_d = nc.dram_tensor("laon", [NV, 4, NPAD], BF16, kind="ExternalInput")
    lh_d = nc.dram_tensor("lh", [NV, 106, 100], BF16, kind="ExternalInput")
    labc_d = nc.dram_tensor("labc", [100, 2 * NP2], F32, kind="ExternalInput")
    o8_d = nc.dram_tensor("o8", [8, 200], BF16, kind="ExternalInput")
    idxv_d = nc.dram_tensor("idxv", [128, HALF], F32, kind="ExternalInput")
    out = nc.dram_tensor("out", [100, NV * 16], F32, kind="ExternalOutput")

    with tile.TileContext(nc) as tc:
        with (
            tc.tile_pool(name="const", bufs=1) as cpool,
            tc.tile_pool(name="stg", bufs=3) as stgp,
            tc.tile_pool(name="lht", bufs=4) as lhp,
            tc.tile_pool(name="t12", bufs=3) as t12p,
            tc.tile_pool(name="th", bufs=3) as thp,
            tc.tile_pool(name="enc", bufs=3) as encp,
            tc.tile_pool(name="ps_a", bufs=2, space="PSUM") as ps_a,
            tc.tile_pool(name="ps_e", bufs=2, space="PSUM") as ps_e,
            tc.tile_pool(name="ps_m", bufs=1, space="PSUM") as ps_m,
        ):
            # --- constants
            o8 = cpool.tile([8, 200], BF16)
            nc.sync.dma_start(o8[:], o8_d[:])
            idxv = cpool.tile([128, HALF], F32)
            nc.sync.dma_start(idxv[:], idxv_d[:])
            labc = cpool.tile([100, 2 * NP2], F32)
            nc.sync.dma_start(labc[:], labc_d[:])
            m8_all = cpool.tile([100, NV * 16], F32)

            for p in range(NP2):
                vA = 2 * p
                # staging: [8, NPAD] = both videos' 4 split rows
                stg = stgp.tile([8, NPAD], BF16)
                nc.sync.dma_start(
                    stg[:], sg4_d[vA:vA + 2].rearrange("v k n -> (v k) n"))

                # th1: rows 0..99 = bf16 head of u; 100..103 = la splits
                # (A_h1, A_h2, B_h1, B_h2); 104..105 = ones
                th1 = thp.tile([106, NPAD], BF16, tag="th1")
                nc.sync.dma_start(th1[100:102, :], laon_d[vA, 0:2])
                nc.sync.dma_start(th1[102:104, :], laon_d[vA + 1, 0:2])
                nc.sync.dma_start(th1[104:106, :], laon_d[vA, 2:4])

                # per-video margin lhsT
                lhA = lhp.tile([106, 100], BF16, tag="lhA")
                lhB = lhp.tile([106, 100], BF16, tag="lhB")
                nc.sync.dma_start(lhA[:], lh_d[vA])
                nc.sync.dma_start(lhB[:], lh_d[vA + 1])

                t1 = t12p.tile([100, NPAD], F32, tag="t1")
                t2 = t12p.tile([100, NPAD], F32, tag="t2")
                for c in range(NPAD // 512):
                    w = 512 if c < 7 else N - 7 * 512
                    cs = slice(c * 512, c * 512 + w)
                    a_ps = ps_a.tile([100, 512], F32)
                    nc.tensor.matmul(a_ps[:, 0:w], o8[:, 0:100], stg[:, cs],
                                     start=True, stop=True)
                    e_ps = ps_e.tile([100, 512], F32)
                    nc.tensor.matmul(e_ps[:, 0:w], o8[:, 100:200], stg[:, cs],
                                     start=True, stop=True)
                    nc.scalar.activation(t1[:, cs], a_ps[:, 0:w],
                                         mybir.ActivationFunctionType.Abs,
                                         bias=labc[:, 2 * p:2 * p + 1],
                                         scale=-1.0)
                    nc.scalar.activation(t2[:, cs], e_ps[:, 0:w],
                                         mybir.ActivationFunctionType.Abs,
                                         bias=labc[:, 2 * p + 1:2 * p + 2],
                                         scale=-1.0)
                # th1 head = bf16(t1 + t2) straight from gpsimd, per half
                for hh in range(2):
                    hs = slice(hh * HALF, HALF + hh * (N - HALF))
                    nc.gpsimd.tensor_tensor(th1[0:100, hs], t1[:, hs],
                                            t2[:, hs], mybir.AluOpType.add)

                for v in range(2):
                    lht = lhA if v == 0 else lhB
                    for h in range(2):
                        hw = HALF if h == 0 else N - HALF
                        mps = ps_m.tile([100, HALF], F32)
                        for cc in range((hw + 511) // 512):
                            w = min(512, hw - cc * 512)
                            ns = slice(h * HALF + cc * 512,
                                       h * HALF + cc * 512 + w)
                            ms = slice(cc * 512, cc * 512 + w)
                            nc.tensor.matmul(mps[:, ms], lht[:],
                                             th1[:, ns],
                                             start=True, stop=True)
                        enc = encp.tile([100, HALF], F32, tag="enc")
                        nc.vector.scalar_tensor_tensor(
                            enc[:, 0:hw], mps[:, 0:hw], 0.0,
                            idxv[0:100, 0:hw],
                            op0=mybir.AluOpType.is_gt,
                            op1=mybir.AluOpType.mult)
                        col = ((p * 2 + v) * 2 + h) * 8
                        nc.vector.max(m8_all[:, col:col + 8], enc[:, 0:hw])
            nc.sync.dma_start(out[:], m8_all[:])
    nc.compile()
    return nc


_NC_CACHE = None


def _get_nc():
    global _NC_CACHE
    if _NC_CACHE is None:
        _NC_CACHE = build_nc()
    return _NC_CACHE


# ------------------------------------------------------------------ host post
NQ = 2
QW = NPAD // NQ


def _decode_first8(dev_out):
    """dev_out: list of [100, NV*32] fp32 per core
    -> cand [B, 2(thr), G, 32] int64, fullq [B, 2, G, NQ]."""
    d = np.stack(dev_out)                                # [C, 100, NV*32]
    d = d.reshape(NCORES, 2, G, NV, NQ, 8)               # [C, t, g, vid, q, s]
    d = np.transpose(d, (0, 3, 1, 2, 4, 5))              # [C, vid, t, g, q, s]
    v = d.reshape(B, 2, G, NQ, 8)
    valid = v > 0.5
    j = np.rint((1.0 - v.astype(np.float64)) * 2.0**19).astype(np.int64) - 1
    off = (np.arange(NQ) * QW)[None, None, None, :, None]
    n = np.where(valid, j + off, -1)
    cand = n.reshape(B, 2, G, NQ * 8)
    cand = np.where((cand >= 0) & (cand < N), cand, -1)
    fullq = valid[:, :, :, :, 7]                         # [b, t, g, q]
    return cand, fullq


def _greedy(cand, fullq, segments, labels):
    """cand [B,2,G,NQ*8] first candidate idxs (-1 none) per (video,thr,GT).
    Returns is_tp [2, B, N] bool. Exact host fallback for >8-deep conflicts."""
    V = B * 2
    c = cand.reshape(V, G, NQ * 8)
    f8 = fullq.reshape(V, G, NQ)
    used = np.zeros((V, N), bool)
    rows = np.arange(V)
    for g in range(G):
        lst = c[:, g, :]                                 # [V, NQ*8]
        valid = lst >= 0
        taken = np.take_along_axis(used, np.clip(lst, 0, None), axis=1)
        free = valid & ~taken
        # fallback: quarter q fully-listed(8) with all its entries
        # used/invalid while deeper candidates may exist there.
        blocked = False
        for q in range(NQ):
            blocked = blocked | (f8[:, g, q]
                                 & ~free[:, 8 * q:8 * q + 8].any(1))
        fb = np.nonzero(blocked)[0]
        for i in fb:
            b, t = i // 2, i % 2
            kinv = np.float32(KINV[t])
            s = segments[b]
            uu = (np.abs(s[:, 0] - labels[b, g, 0])
                  + np.abs(s[:, 1] - labels[b, g, 1])).astype(np.float32)
            marg = ((s[:, 1] - s[:, 0]) +
                    (labels[b, g, 1] - labels[b, g, 0]) - kinv * uu)
            cnz = np.nonzero(marg.astype(np.float32) > 0)[0]
            fr = cnz[~used[i, cnz]]
            if len(fr):
                used[i, fr[0]] = True
            free[i] = False
        has = free.any(1)
        j = np.argmax(free, axis=1)
        sel = rows[has]
        used[sel, lst[sel, j[has]]] = True
    return used.reshape(B, 2, N).transpose(1, 0, 2)


def _ap_from_tp(is_tp, scores):
    """is_tp [2, B, N] bool, scores [B, N] -> AP [2] float32 (exact ranking)."""
    conf = scores.reshape(-1)
    M = conf.size
    bits = conf.view(np.uint32).astype(np.int64)
    key = (bits << 20) + (2**20 - 1 - np.arange(M, dtype=np.int64))
    skey = np.sort(key)
    out = np.empty(2, np.float32)
    for t in range(2):
        tp_idx = np.nonzero(is_tp[t].reshape(-1))[0]
        k = key[tp_idx]
        # rank (1-based) in descending order = #{keys > k} + 1
        r = np.sort(M - np.searchsorted(skey, k, side="left"))
        kk = np.arange(1, len(r) + 1, dtype=np.float64)
        prec = (kk / r).astype(np.float32)
        sufmax = np.maximum.accumulate(prec[::-1])[::-1]
        out[t] = np.float32(sufmax.astype(np.float64).sum() / (B * G))
    return out


def _enable_profiling():
    """Dev-only: register the NTFF profiling hook (missing antenv shim) and
    keep artifacts local. Returns extra kwargs for run_bass_kernel_spmd."""
    import sys
    import types
    import tempfile

    if "antenv.axon_hooks" not in sys.modules:
        mod = types.ModuleType("antenv.axon_hooks")
        _h = [None]
        mod.set_axon_ntff_profile_hook = lambda h: _h.__setitem__(0, h)
        mod.get_axon_ntff_profile_hook = lambda: _h[0]
        sys.modules["antenv.axon_hooks"] = mod
        from trn_agent_boot.trn_boot import _ntff_profile_via_ctypes
        mod.set_axon_ntff_profile_hook(
            _ntff_profile_via_ctypes("/opt/axon/libaxon_pjrt.so"))
    import concourse.bass_utils as bu
    bu.upload_artifacts = lambda tmpdir: tmpdir
    tdir = os.environ.get("ATH_TRACE_DIR") or tempfile.mkdtemp(
        prefix="ap_trace_")
    print("trace dir:", tdir)
    return {"tmpdir": tdir}


# ------------------------------------------------------------------- kernel
def kernel(scores, segments, labels):
    scores = np.ascontiguousarray(scores, np.float32)
    segments = np.ascontiguousarray(segments, np.float32)
    labels = np.ascontiguousarray(labels, np.float32)

    o8, idxval = _consts()
    in_maps = []
    for i in range(NCORES):
        sl = slice(i * NV, (i + 1) * NV)
        m = _core_inputs(segments[sl], labels[sl])
        m.update({"o8": o8, "idxv": idxval})
        in_maps.append(m)
    nc = _get_nc()
    trace = bool(int(os.environ.get("ATH_PROFILE", "0")))
    kw = {}
    if trace:
        try:
            kw = _enable_profiling()
        except Exception as e:           # profiling is best-effort
            print("profiling unavailable:", e)
            trace = False
    res = run_bass_kernel_spmd(nc, in_maps, core_ids=list(range(NCORES)),
                               trace=trace, **kw)
    if trace and res.exec_time_ns is not None:
        print(f"HW exec time: {res.exec_time_ns} ns")
    dev = [res.results[i]["out"] for i in range(NCORES)]

    cand, full8 = _decode_first8(dev)
    is_tp = _greedy(cand, full8, segments, labels)
    return _ap_from_tp(is_tp, scores)
